# revision 1
# baseline (speedup 1.0000x reference)
"""AltConv via Winograd F(3,4) on 8 TRN2 NeuronCores.

out[s] = sum_{i=0..3} K_i x[s-i].  Outputs in triples (3u, 3u+1, 3u+2) from 6
Winograd-channel matmuls instead of 12 (2x fewer PE cycles than direct):

  w_l(u) = x[3u-3+l], l=0..5
  x~_j = sum_l BT[j,l] w_l     (host)        K~_j = sum_i G[j,i] K_{3-i} (host)
  P_j  = x~_j @ K~_j           (device TensorE, f32 accum over D)
  out[3u]   = P0 + P1 + P2 + P3 + P4
  out[3u+1] = P1 - P2 + 2 P3 - 0.5 P4
  out[3u+2] = P1 + P2 + 4 P3 + 0.25 P4 + P5
  (points {0, 1, -1, 2, -1/2, inf}; measured bf16 rel err ~6.2e-3)

Sharding: data-parallel over (batch, seq-half) -> 8 shards of 4096 tokens,
U = 1366 output triples each (2 pad tokens dropped on host).  ScalarE seeds
each output plane from PSUM, VectorE accumulates the rest, GpSimd issues the
output stores so input loads never queue behind them.
"""

import numpy as np
import ml_dtypes

B, S, D, F, R = 4, 8192, 1024, 1024, 4
N_CORES = 8
T = S // 2            # tokens per core
M = 3                 # outputs per Winograd block
NJ = 6                # Winograd channels
KD = D // 128
FB = F // 128
U3 = (T + M - 1) // M + 1   # 1366 blocks (covers 4098 tokens)
CHUNKS = [(0, 512), (512, 512), (1024, 342)]   # (u start, width)

_POINTS = [0.0, 1.0, -1.0, 2.0, -0.5]
_CACHE = {}


def _mats():
    n, r = NJ, R
    G = np.zeros((n, r))
    for j, p in enumerate(_POINTS):
        G[j] = [p ** e for e in range(r)]
    G[-1, r - 1] = 1.0
    V = np.zeros((n, n))
    for j, p in enumerate(_POINTS):
        V[j] = [p ** e for e in range(n)]
    V[-1, -1] = 1.0
    BT = np.linalg.inv(V).T
    return G, BT


def _build():
    if "nc" in _CACHE:
        return _CACHE["nc"]
    import concourse.tile as tile
    from concourse import bacc, mybir

    nc = bacc.Bacc("TRN2", target_bir_lowering=False, debug=False,
                   num_devices=N_CORES)
    bf16 = mybir.dt.bfloat16
    f32 = mybir.dt.float32

    # DRAM layouts pre-swizzled on host to match SBUF tiles
    xt_ds = [nc.dram_tensor(f"xt{c}", [128, NJ, KD, w], bf16,
                            kind="ExternalInput")
             for c, (_, w) in enumerate(CHUNKS)]
    kt_d = nc.dram_tensor("kt", [FB, 128, NJ, KD, 128], bf16,
                          kind="ExternalInput")
    out_d = nc.dram_tensor("outT", [FB, 128, M, U3], f32,
                           kind="ExternalOutput")

    with tile.TileContext(nc) as tc:
        with (
            tc.tile_pool(name="kpool", bufs=1) as kpool,
            tc.tile_pool(name="xpool", bufs=2) as xpool,
            tc.tile_pool(name="psum", bufs=1, space="PSUM") as ppool,
            tc.tile_pool(name="stage", bufs=2) as spool,
        ):
            kt = kpool.tile([128, FB, NJ, KD, 128], bf16)
            for c, (lo, w) in enumerate(CHUNKS):
                xt = xpool.tile([128, NJ, KD, w], bf16, name=f"xt{c}",
                                tag="xt")
                if c == 0:
                    # fine-grained first loads so the PE can start early
                    for kdh in range(4):
                        ks = slice(kdh * 2, (kdh + 1) * 2)
                        nc.sync.dma_start(kt[:, 0, :, ks, :],
                                          kt_d[0, :, :, ks, :])
                        nc.sync.dma_start(xt[:, :, ks, :],
                                          xt_ds[c][:, :, ks, :])
                    for fb in range(1, FB):
                        for kdh in range(2):
                            ks = slice(kdh * 4, (kdh + 1) * 4)
                            nc.sync.dma_start(kt[:, fb, :, ks, :],
                                              kt_d[fb, :, :, ks, :])
                else:
                    for kdh in range(4):
                        ks = slice(kdh * 2, (kdh + 1) * 2)
                        nc.sync.dma_start(xt[:, :, ks, :],
                                          xt_ds[c][:, :, ks, :])
                for fb in range(FB):
                    # pb0/pb1 double-buffered (2 spare banks) so the next
                    # block's first matmuls never wait on the combine
                    Ps = [ppool.tile([128, 512], f32, tag=f"pb{j}",
                                     name=f"P{c}_{fb}_{j}",
                                     bufs=2 if j < 2 else 1)
                          for j in range(NJ)]
                    for kd in range(KD):
                        for j in range(NJ):
                            nc.tensor.matmul(
                                Ps[j][:, :w],
                                kt[:, fb, j, kd, :],
                                xt[:, j, kd, :],
                                start=(kd == 0), stop=(kd == KD - 1),
                            )
                    st = spool.tile([128, M, 512], f32, tag="st")
                    t0, t1, t2 = (st[:, t, :w] for t in range(M))
                    P = [p[:, :w] for p in Ps]
                    mult = mybir.AluOpType.mult
                    add = mybir.AluOpType.add
                    # reads ordered by source bank so P_j banks retire early
                    nc.scalar.copy(t0, P[0])
                    nc.scalar.mul(t1, P[3], 2.0)
                    nc.scalar.mul(t2, P[3], 4.0)
                    nc.vector.tensor_add(t0, t0, P[1])
                    nc.vector.tensor_add(t1, t1, P[1])
                    nc.vector.tensor_add(t2, t2, P[1])
                    nc.vector.tensor_add(t0, t0, P[2])
                    nc.vector.tensor_sub(t1, t1, P[2])
                    nc.vector.tensor_add(t2, t2, P[2])
                    nc.vector.tensor_add(t0, t0, P[3])
                    nc.vector.tensor_add(t0, t0, P[4])
                    nc.vector.scalar_tensor_tensor(t1, P[4], -0.5, t1,
                                                   mult, add)
                    nc.vector.scalar_tensor_tensor(t2, P[4], 0.25, t2,
                                                   mult, add)
                    nc.vector.tensor_add(t2, t2, P[5])
                    nc.gpsimd.dma_start(out_d[fb, :, :, lo:lo + w],
                                        st[:, :, :w])

    nc.compile()
    _CACHE["nc"] = nc
    return nc


def _prep_inputs(x, kernels):
    bf16 = ml_dtypes.bfloat16
    G, BT = _mats()
    Kt = np.einsum("ji,idf->jdf", G, kernels[::-1].astype(np.float64))
    kt_bf = np.ascontiguousarray(
        Kt.reshape(NJ, KD, 128, FB, 128).transpose(3, 2, 0, 1, 4).astype(bf16))
    in_maps = []
    for c in range(N_CORES):
        b, h = divmod(c, 2)
        # w_l(u) = x[b, h*T + 3u - 3 + l], zeros outside [h*T, h*T+T)
        need = M * (U3 - 1) + NJ            # 4101 padded rows
        xp = np.zeros((need, D), dtype=np.float32)
        s0 = h * T - 3
        lo, hi = max(s0, 0), min(s0 + need, S)
        xp[lo - s0: hi - s0] = x[b, lo: hi]
        if h == 1:                          # zero anything past this shard
            over = (s0 + need) - (h * T + T)
            if over > 0:
                xp[need - over:] = 0.0
        idx = M * np.arange(U3)
        W6 = np.stack([xp[idx + l] for l in range(NJ)])      # [6, U3, D]
        Xt = np.einsum("jl,lud->jud", BT, W6).astype(bf16)   # [6, U3, D]
        Xr = Xt.reshape(NJ, U3, KD, 128).transpose(3, 0, 2, 1)  # [dp,j,kd,u]
        shard = {"kt": kt_bf}
        for ci, (ulo, w) in enumerate(CHUNKS):
            shard[f"xt{ci}"] = np.ascontiguousarray(Xr[:, :, :, ulo:ulo + w])
        in_maps.append(shard)
    return in_maps


def kernel(x, kernels, biases, trace=False):
    from concourse.bass_utils import run_bass_kernel_spmd

    x = np.asarray(x, dtype=np.float32)
    kernels = np.asarray(kernels, dtype=np.float32)
    biases = np.asarray(biases, dtype=np.float32)
    nc = _build()
    in_maps = _prep_inputs(x, kernels)
    res = run_bass_kernel_spmd(nc, in_maps, core_ids=list(range(N_CORES)),
                               trace=trace)
    out = np.empty((B, S, F), dtype=np.float32)
    for c in range(N_CORES):
        b, h = divmod(c, 2)
        o = res.results[c]["outT"]          # [FB, 128, M, U3]
        for t in range(M):
            cnt = (T - t + M - 1) // M
            out[b, h * T + t:(h + 1) * T:M, :] = \
                o[:, :, t, :cnt].reshape(F, cnt).T
    bias_total = biases.astype(np.float32).sum(axis=0)
    if np.any(bias_total):
        out += bias_total
    if trace:
        kernel.last_exec_time_ns = res.exec_time_ns
    return out



# revision 3
# speedup vs baseline: 1.2340x; 1.2340x over previous
"""AltConv via Winograd F(8,4) in fp16 on 8 TRN2 NeuronCores.

out[s] = sum_{i=0..3} K_i x[s-i].  Outputs in blocks of M=8 from NJ=11
Winograd-channel matmuls instead of 32 (2.9x fewer PE cycles than direct):

  w_l(u) = x[8u-3+l], l=0..10
  x~_j = alpha_j (BT w)_j   (host f64)    K~_j = beta_j (G Krev)_j (host f64)
  P_j  = x~_j @ K~_j        (device TensorE fp16, f32 PSUM accum over D)
  out[8u+t] = sum_j A[t,j]/(alpha_j beta_j) P_j(u)   (DVE fp16 chain)

Points {0, +-1, +-a, +-b, +-c, d, inf} (numerically optimized); per-channel
scales keep every fp16 plane in range.  Sim-predicted rel err ~8.4e-3.

Sharding: data-parallel over (batch, seq-half) -> 8 shards of T=4096 tokens,
U=512 output blocks each (exact fit, one PSUM bank per P plane).  Per fb
(128 features): 11 channels x 8 kd accumulate sequentially into a rotating
4-bank PSUM pool; ScalarE drains each plane to fp16 SBUF; DVE combines
pairs (S/D) then per-t scalar_tensor_tensor chains; GpSimd stores fp16.
"""

import numpy as np

B, S, D, F, R = 4, 8192, 1024, 1024, 4
N_CORES = 8
T = S // 2            # tokens per core
M = 8                 # outputs per Winograd block
NJ = M + R - 1        # 11 Winograd channels
KD = D // 128
FB = F // 128
U = T // M            # 512 blocks per core (exact)

# optimized points: pairs (1, a, b, c), single d, zero, inf
PAIR_VALS = [1.0, 0.3744, 0.7256, 1.6749]
SINGLES = [4.0762]
_CACHE = {}


def _mats():
    pts = []
    for p in PAIR_VALS:
        pts += [p, -p]
    pts += list(SINGLES) + [0.0]
    n = NJ
    G = np.zeros((n, R))
    for j, p in enumerate(pts):
        G[j] = [p ** e for e in range(R)]
    G[-1, R - 1] = 1.0
    V = np.zeros((n, n))
    for j, p in enumerate(pts):
        V[j] = [p ** e for e in range(n)]
    V[-1, -1] = 1.0
    BT = np.linalg.inv(V).T
    A = np.zeros((M, n))
    for j, p in enumerate(pts):
        A[:, j] = [p ** t for t in range(M)]
    A[M - 1, n - 1] = 1.0
    alpha = 1.0 / np.linalg.norm(BT, axis=1)
    beta = 64.0 / np.linalg.norm(G, axis=1)
    for i in range(len(PAIR_VALS)):
        alpha[2 * i] = alpha[2 * i + 1] = min(alpha[2 * i], alpha[2 * i + 1])
        beta[2 * i] = beta[2 * i + 1] = min(beta[2 * i], beta[2 * i + 1])
    beta[0] = beta[1] = 1.0 / alpha[0]      # base of the stt chain: A' = +-1
    Ap = A / (alpha * beta)[None, :]
    return G, BT, A, Ap, alpha, beta


def _build():
    if "nc" in _CACHE:
        return _CACHE["nc"]
    import concourse.tile as tile
    from concourse import bacc, mybir

    _, _, _, Ap, _, _ = _mats()
    nc = bacc.Bacc("TRN2", target_bir_lowering=False, debug=False,
                   num_devices=N_CORES)
    f16 = mybir.dt.float16
    f32 = mybir.dt.float32
    mult = mybir.AluOpType.mult
    add = mybir.AluOpType.add

    xt_d = nc.dram_tensor("xt", [128, NJ, KD, U], f16, kind="ExternalInput")
    kt_d = nc.dram_tensor("kt", [FB, 128, NJ, KD, 128], f16,
                          kind="ExternalInput")
    out_d = nc.dram_tensor("outT", [FB, 128, M, U], f16,
                           kind="ExternalOutput")
    NPAIR = len(PAIR_VALS)
    J_SINGLE = 2 * NPAIR          # 8
    J_ZERO = NJ - 2               # 9
    J_INF = NJ - 1                # 10

    with tile.TileContext(nc) as tc:
        with (
            tc.tile_pool(name="xpool", bufs=1) as xpool,
            tc.tile_pool(name="kpool", bufs=1) as kpool,
            tc.tile_pool(name="psum", bufs=1, space="PSUM") as ppool,
            tc.tile_pool(name="stage", bufs=1) as spool,
        ):
            xt = xpool.tile([128, NJ, KD, U], f16)

            for fb in range(FB):
                kt = kpool.tile([128, NJ, KD, 128], f16, name=f"kt{fb}",
                                tag="kt", bufs=2)
                if fb == 0:
                    # fine-grained, kt slice before xt slice per channel,
                    # so the first matmuls start as soon as channel 0 lands
                    for j in range(NJ):
                        nc.sync.dma_start(kt[:, j, :, :], kt_d[fb, :, j, :, :])
                        nc.sync.dma_start(xt[:, j, :, :], xt_d[:, j, :, :])
                else:
                    for g in range(2):
                        js = slice(g * 6, min((g + 1) * 6, NJ))
                        nc.sync.dma_start(kt[:, js, :, :],
                                          kt_d[fb, :, js, :, :])

                pc = []
                for j in range(NJ):
                    P = ppool.tile([128, U], f32, tag="P", bufs=4,
                                   name=f"P{fb}_{j}")
                    for kd in range(KD):
                        nc.tensor.matmul(
                            P, kt[:, j, kd, :], xt[:, j, kd, :],
                            start=(kd == 0), stop=(kd == KD - 1),
                        )
                    c = spool.tile([128, U], f16, tag=f"pc{j}", bufs=2,
                                   name=f"pc{fb}_{j}")
                    nc.scalar.copy(c, P)
                    pc.append(c)

                sd = {}
                for i in range(NPAIR):
                    s_ = spool.tile([128, U], f16, tag=f"sd{i}S", bufs=2,
                                    name=f"S{fb}_{i}")
                    d_ = spool.tile([128, U], f16, tag=f"sd{i}D", bufs=2,
                                    name=f"D{fb}_{i}")
                    nc.vector.tensor_add(s_, pc[2 * i], pc[2 * i + 1])
                    nc.vector.tensor_sub(d_, pc[2 * i], pc[2 * i + 1])
                    sd[(i, 0)] = s_
                    sd[(i, 1)] = d_

                st = spool.tile([128, M, U], f16, tag="st", bufs=2,
                                name=f"st{fb}")
                for t in range(M):
                    acc = st[:, t, :]
                    par = t % 2
                    nc.vector.scalar_tensor_tensor(
                        acc, sd[(1, par)], float(Ap[t, 2]), sd[(0, par)],
                        mult, add)
                    for i in (2, 3):
                        nc.vector.scalar_tensor_tensor(
                            acc, sd[(i, par)], float(Ap[t, 2 * i]), acc,
                            mult, add)
                    nc.vector.scalar_tensor_tensor(
                        acc, pc[J_SINGLE], float(Ap[t, J_SINGLE]), acc,
                        mult, add)
                    if t == 0:
                        nc.vector.scalar_tensor_tensor(
                            acc, pc[J_ZERO], float(Ap[0, J_ZERO]), acc,
                            mult, add)
                    if t == M - 1:
                        nc.vector.scalar_tensor_tensor(
                            acc, pc[J_INF], float(Ap[M - 1, J_INF]), acc,
                            mult, add)
                nc.gpsimd.dma_start(out_d[fb, :, :, :], st[:, :, :])

    nc.compile()
    _CACHE["nc"] = nc
    return nc


def _prep_inputs(x, kernels):
    G, BT, _, _, alpha, beta = _mats()
    Kt = np.einsum("ji,idf->jdf", G, kernels[::-1].astype(np.float64)) \
        * beta[:, None, None]
    kt16 = np.ascontiguousarray(
        Kt.reshape(NJ, KD, 128, FB, 128).transpose(3, 2, 0, 1, 4)
    ).astype(np.float16)
    need = M * (U - 1) + NJ               # 4099 padded rows
    idx = M * np.arange(U)
    in_maps = []
    for c in range(N_CORES):
        b, h = divmod(c, 2)
        xp = np.zeros((need, D), dtype=np.float64)
        s0 = h * T - (R - 1)
        lo = max(s0, 0)
        xp[lo - s0: need] = x[b, lo: s0 + need]
        Wn = np.stack([xp[idx + l] for l in range(NJ)])      # [11, U, D]
        Xt = np.einsum("jl,lud->jud", BT, Wn) * alpha[:, None, None]
        Xr = Xt.reshape(NJ, U, KD, 128).transpose(3, 0, 2, 1)
        in_maps.append({"kt": kt16,
                        "xt": np.ascontiguousarray(Xr).astype(np.float16)})
    return in_maps


def kernel(x, kernels, biases, trace=False):
    from concourse.bass_utils import run_bass_kernel_spmd

    x = np.asarray(x, dtype=np.float32)
    kernels = np.asarray(kernels, dtype=np.float32)
    biases = np.asarray(biases, dtype=np.float32)
    nc = _build()
    in_maps = _prep_inputs(x, kernels)
    res = run_bass_kernel_spmd(nc, in_maps, core_ids=list(range(N_CORES)),
                               trace=trace)
    out = np.empty((B, S, F), dtype=np.float32)
    for c in range(N_CORES):
        b, h = divmod(c, 2)
        o = res.results[c]["outT"]            # [FB, 128, M, U] fp16
        out[b, h * T:(h + 1) * T] = (
            o.transpose(3, 2, 0, 1).reshape(T, F).astype(np.float32))
    bias_total = biases.astype(np.float32).sum(axis=0)
    if np.any(bias_total):
        out += bias_total
    if trace:
        kernel.last_exec_time_ns = res.exec_time_ns
    return out
